# revision 32
# baseline (speedup 1.0000x reference)
"""Grouped-experts SwiGLU FFN (MoE) on 8 Trainium2 NeuronCores.

Expert-parallel: core e owns expert e's weights and its contiguous token
slice.  Tokens are already sorted by expert (contiguous ranges from
cumsum(num_tokens_per_expert)), so the "all-to-all dispatch" is plain host
slicing.  Each core runs a two-stage SwiGLU:

  stage 1:  HT[h, t] = silu(W1 x)[h, t] * (W3 x)[h, t]     (K = DIM)
  stage 2:  OUT.T[d, t] = (W2 @ H)[d, t]                   (K = HIDDEN)

Matmuls run in bf16 (1 cycle/row on the PE array, fp32 PSUM accumulate);
~4e-3 relative error vs the fp32 reference.  Host pre-packs x and weights
into SBUF-tile layout [128, ...] so every DMA reads contiguous lines; the
kernel returns OUT.T per core and the host transposes/scatters back.

fp8 DoubleRow (2x PE throughput) everywhere was evaluated and rejected:
e4m3's 3-bit mantissa gives ~2.6% per-element quant error and the
three-matmul chain lands at ~6.5% rel err vs the 2e-2 budget.  But a
SURGICAL fp8 slice fits: 2 of stage 2's 11 contraction blocks run as one
DoubleRow matmul (512 streaming cycles -> ~578/2), saving ~190ns per
output tile; exact-data sim says rel err 1.64e-2 < 2e-2.  Scales are
folded into host weight packing (w3 blocks 0,1 x4 so the DVE mul emits
e4m3(4h); w3 blocks 2..10 x128 so bf16 ht carries 128h; fp8 w2 blocks
x32 so both PSUM partials are 128x the true value; one scaled Copy
drains /128).
"""

import numpy as np
import ml_dtypes

import concourse.bass as bass
from concourse import bacc
import concourse.mybir as mybir
from concourse.tile import TileContext
from concourse.bass_utils import run_bass_kernel_spmd

N_TOKENS = 16384
DIM = 2048
HIDDEN = 1408
N_EXPERTS = 8
N_CORES = 8

P = 128
T = 2048                 # token capacity per core per pass
N_DN = DIM // P          # 16 contraction blocks in stage 1
N_HT = HIDDEN // P       # 11 h tiles
N_F8 = 2                 # h-blocks 0..1 run stage 2 in fp8 DoubleRow
N_HB = N_HT - N_F8       # h-blocks 2..10 run stage 2 in bf16
N_DT = DIM // P          # 16 output-row tiles in stage 2
TSUB = 512               # moving-operand width per matmul (1 PSUM bank)

F32 = mybir.dt.float32
BF16 = mybir.dt.bfloat16
F8E4 = mybir.dt.float8e4
SILU = mybir.ActivationFunctionType.Silu
COPY = mybir.ActivationFunctionType.Copy
DROW = mybir.MatmulPerfMode.DoubleRow
BF = ml_dtypes.bfloat16
F8 = ml_dtypes.float8_e4m3
H8_SCALE = 4.0           # ht8 = e4m3(4h); |4h|max ~62 << 240
W28_SCALE = 32.0         # w2 fp8 blocks x32; |32 w2|max ~3.5
HT_SCALE = 128.0         # bf16 ht carries 128h; PSUM holds 128*out


def _build_program() -> bass.Bass:
    nc = bacc.Bacc()
    xtp = nc.declare_dram_parameter(
        "xtp", [P, T // TSUB, N_DN, TSUB], BF16, isOutput=False)
    w1p = nc.declare_dram_parameter("w1p", [P, N_HT, N_DN, P], BF16, isOutput=False)
    w3p = nc.declare_dram_parameter("w3p", [P, N_HT, N_DN, P], BF16, isOutput=False)
    w2p = nc.declare_dram_parameter("w2p", [P, N_DT, N_HB, P], BF16, isOutput=False)
    w28p = nc.declare_dram_parameter("w28p", [P, N_DT, N_F8, P], F8E4, isOutput=False)
    outt = nc.declare_dram_parameter("outt", [DIM, T], F32, isOutput=True)

    with TileContext(nc) as tc:
        with (
            tc.tile_pool(name="xt", bufs=1) as xt_pool,
            tc.tile_pool(name="ht", bufs=1) as ht_pool,
            tc.tile_pool(name="w1", bufs=3) as w1_pool,
            tc.tile_pool(name="w3", bufs=3) as w3_pool,
            tc.tile_pool(name="w2", bufs=1) as w2_pool,
            tc.tile_pool(name="w28", bufs=1) as w28_pool,
            tc.tile_pool(name="tmp", bufs=3) as tmp_pool,
            tc.tile_pool(name="ob", bufs=2) as ob_pool,
            tc.tile_pool(name="ps", bufs=1, space="PSUM") as ps_pool,
        ):
            # DMA queue order = program order: quarter-loads of
            # (w1, x-chunk0, w3) first so the first matmuls' operands land
            # first, then the later x chunks also in quarters -- the PE
            # consumes them progressively (a whole-chunk load measurably
            # stalls the PE ~3us waiting for the full 2MB)
            w1b0 = w1_pool.tile([P, N_DN, P], BF16, tag="w1")
            w3b0 = w3_pool.tile([P, N_DN, P], BF16, tag="w3")
            xcs = [
                xt_pool.tile([P, N_DN, TSUB], BF16, bufs=T // TSUB,
                             tag="xt", name=f"xc{i}")
                for i in range(T // TSUB)
            ]
            # x-chunk0 quarters issue from the Scalar engine's own HW-DGE
            # ring (qActDynamicHW), in parallel with the weight quarters on
            # the Sync ring (qSPDynamicHW): halves the serialized ~0.6us/
            # dma_start descriptor programming on the first matmul's
            # critical path.  Scalar's queue is otherwise empty until the
            # first silu at ~14us.
            for q in range(4):
                dn = slice(q * 4, (q + 1) * 4)
                nc.sync.dma_start(out=w1b0[:, dn, :], in_=w1p[:, 0, dn, :])
                nc.scalar.dma_start(out=xcs[0][:, dn, :], in_=xtp[:, 0, dn, :])
                nc.sync.dma_start(out=w3b0[:, dn, :], in_=w3p[:, 0, dn, :])
            # later x chunks also on the scalar ring: it is idle after
            # x-chunk0, while the sync ring still carries every weight
            # load -- keeping x off it bought the first matmul ~1-3us and
            # moving xc1..3 here removes the ~2.7us its=1 stall that the
            # earlier head start exposed
            for c in range(1, T // TSUB):
                for q in range(4):
                    dn = slice(q * 4, (q + 1) * 4)
                    nc.scalar.dma_start(out=xcs[c][:, dn, :],
                                        in_=xtp[:, c, dn, :])
            xts = xcs
            ht = ht_pool.tile([P, N_HB, T], BF16)
            ht8 = ht_pool.tile([P, N_F8, T], F8E4, name="ht8")

            # stage 1: HT[h, t] = silu(x @ w1.T).T * (x @ w3.T).T
            # (host pre-scaled w3 so blocks 0..1 emit e4m3(4h) and blocks
            # 2..10 emit bf16(128h) straight out of the DVE mul)
            for ih in range(N_HT):
                if ih == 0:
                    w1b, w3b = w1b0, w3b0
                else:
                    w1b = w1_pool.tile([P, N_DN, P], BF16, tag="w1")
                    nc.sync.dma_start(out=w1b[:], in_=w1p[:, ih, :, :])
                    w3b = w3_pool.tile([P, N_DN, P], BF16, tag="w3")
                    nc.sync.dma_start(out=w3b[:], in_=w3p[:, ih, :, :])
                for its in range(T // TSUB):
                    ts0 = its * TSUB
                    xt_c = xts[its]
                    ps1 = ps_pool.tile([P, TSUB], F32, bufs=2, name="ps1")
                    ps2 = ps_pool.tile([P, TSUB], F32, bufs=2, name="ps2")
                    for n in range(N_DN):
                        nc.tensor.matmul(
                            ps1[:],
                            lhsT=w1b[:, n, :],
                            rhs=xt_c[:, n, :],
                            start=(n == 0),
                            stop=(n == N_DN - 1),
                        )
                    for n in range(N_DN):
                        nc.tensor.matmul(
                            ps2[:],
                            lhsT=w3b[:, n, :],
                            rhs=xt_c[:, n, :],
                            start=(n == 0),
                            stop=(n == N_DN - 1),
                        )
                    tmp = tmp_pool.tile([P, TSUB], F32)
                    nc.scalar.activation(tmp[:], ps1[:], SILU)
                    if ih < N_F8:
                        dst = ht8[:, ih, ts0:ts0 + TSUB]
                    else:
                        dst = ht[:, ih - N_F8, ts0:ts0 + TSUB]
                    nc.vector.tensor_mul(dst, tmp[:], ps2[:])

            # stage 2: OUT.T[d, t] = sum_h W2T[h, d] * HT[h, t]
            # h-blocks 0..1 as one fp8 DoubleRow matmul per (idt, its)
            # (PSUM partial is (4h)*(32 w2) = 128*true, matching the bf16
            # partials (128h)*w2); drain divides by 128.  All w2 weights
            # load as 2 whole-tensor DMAs (SBUF has room; kills 32 of the
            # ~0.6us dma_start programming slots on the Sync engine).
            w2s = w2_pool.tile([P, N_DT, N_HB, P], BF16)
            nc.sync.dma_start(out=w2s[:], in_=w2p[:])
            w28s = w28_pool.tile([P, N_DT, N_F8, P], F8E4)
            nc.sync.dma_start(out=w28s[:], in_=w28p[:])
            NTS = T // TSUB
            for idt in range(N_DT):
                # ts-major: one PSUM bank accumulates DR + 9 bf16, drains
                # via a scaled Copy on ScalarE while the next chunk streams;
                # bank rotation happens once per 10 matmuls (per-MM rotation
                # measurably costs ~2.5us in PE micro-gaps)
                ob = ob_pool.tile([P, T], F32)
                for its in range(NTS):
                    seg = slice(its * TSUB, (its + 1) * TSUB)
                    pso = ps_pool.tile([P, TSUB], F32, bufs=4, name="pso")
                    nc.tensor.matmul(
                        pso[:],
                        lhsT=w28s[:, idt, :, :],
                        rhs=ht8[:, :, seg],
                        start=True,
                        stop=False,
                        perf_mode=DROW,
                    )
                    for hb in range(N_HB):
                        nc.tensor.matmul(
                            pso[:],
                            lhsT=w2s[:, idt, hb, :],
                            rhs=ht[:, hb, seg],
                            start=False,
                            stop=(hb == N_HB - 1),
                        )
                    if idt < N_DT - 1:
                        nc.scalar.activation(ob[:, seg], pso[:],
                                             COPY, scale=1.0 / HT_SCALE)
                    elif its < NTS - 1:
                        # last tile: chunks drain individually right away
                        # so only the final chunk remains after the last MM
                        nc.scalar.activation(ob[:, seg], pso[:],
                                             COPY, scale=1.0 / HT_SCALE)
                        nc.sync.dma_start(
                            out=outt[idt * P:(idt + 1) * P, seg],
                            in_=ob[:, seg])
                    else:
                        # final chunk split in two so the exposed tail is
                        # only act(256) + program + dma(256)
                        for h in range(2):
                            lo = its * TSUB + h * (TSUB // 2)
                            hi = lo + TSUB // 2
                            nc.scalar.activation(
                                ob[:, lo:hi],
                                pso[:, h * (TSUB // 2):(h + 1) * (TSUB // 2)],
                                COPY, scale=1.0 / HT_SCALE)
                            nc.sync.dma_start(
                                out=outt[idt * P:(idt + 1) * P, lo:hi],
                                in_=ob[:, lo:hi])
                if idt < N_DT - 1:
                    # one batched drain DMA per output-row tile
                    nc.sync.dma_start(out=outt[idt * P:(idt + 1) * P, :],
                                      in_=ob[:])
    nc.compile()
    return nc


_CACHE: dict = {}


def _get_nc() -> bass.Bass:
    if "nc" not in _CACHE:
        _CACHE["nc"] = _build_program()
    return _CACHE["nc"]


def _pack_weights(w1, w2, w3):
    maps = []
    for e in range(N_EXPERTS):
        w3s = w3[e].copy()
        w3s[:N_F8 * P] *= H8_SCALE
        w3s[N_F8 * P:] *= HT_SCALE
        w2bf = w2[e][:, N_F8 * P:]
        w28 = np.clip(w2[e][:, :N_F8 * P] * W28_SCALE, -240.0, 240.0)
        maps.append({
            "w1p": np.ascontiguousarray(
                w1[e].reshape(N_HT, P, N_DN, P).transpose(3, 0, 2, 1).astype(BF)),
            "w3p": np.ascontiguousarray(
                w3s.reshape(N_HT, P, N_DN, P).transpose(3, 0, 2, 1).astype(BF)),
            "w2p": np.ascontiguousarray(
                w2bf.reshape(N_DT, P, N_HB, P).transpose(3, 0, 2, 1).astype(BF)),
            "w28p": np.ascontiguousarray(
                w28.reshape(N_DT, P, N_F8, P).transpose(3, 0, 2, 1).astype(F8)),
        })
    return maps


def kernel(x, w1, w2, w3, num_tokens_per_expert, _trace=False):
    x = np.ascontiguousarray(np.asarray(x, dtype=np.float32))
    w1 = np.ascontiguousarray(np.asarray(w1, dtype=np.float32))
    w2 = np.ascontiguousarray(np.asarray(w2, dtype=np.float32))
    w3 = np.ascontiguousarray(np.asarray(w3, dtype=np.float32))
    counts = np.asarray(num_tokens_per_expert, dtype=np.int64)

    cs = np.cumsum(counts)
    starts = np.minimum(np.concatenate([[0], cs[:-1]]), N_TOKENS)
    ends = np.minimum(cs, N_TOKENS)
    lens = np.maximum(ends - starts, 0)

    wmaps = _pack_weights(w1, w2, w3)
    out = np.zeros((N_TOKENS, DIM), np.float32)
    trace_info = []

    n_passes = max(1, int(np.max(np.ceil(lens / T))))
    for k in range(n_passes):
        in_maps = []
        for e in range(N_EXPERTS):
            s = int(starts[e]) + k * T
            xe = np.zeros((T, DIM), np.float32)
            avail = x[s:s + T]
            if avail.shape[0]:
                xe[:avail.shape[0]] = avail
            # [P, n_chunks, N_DN, TSUB]: xtp[p, c, n, t] = x[c*TSUB+t, n*128+p]
            xtp = np.ascontiguousarray(
                xe.T.reshape(N_DN, P, T // TSUB, TSUB)
                .transpose(1, 2, 0, 3).astype(BF))
            in_maps.append({"xtp": xtp, **wmaps[e]})
        res = run_bass_kernel_spmd(
            _get_nc(), in_maps, list(range(N_CORES)), trace=_trace
        )
        if _trace:
            trace_info.append(res)
        for e in range(N_EXPERTS):
            s = int(starts[e]) + k * T
            cnt = min(int(ends[e]) - s, T)
            if cnt > 0:
                out[s:s + cnt] = res.results[e]["outt"].T[:cnt]

    if _trace:
        return out, trace_info
    return out

